# revision 35
# baseline (speedup 1.0000x reference)
"""Bahdanau attention kernel for Trainium2 (8 NeuronCores, data-parallel over batch).

Reference computation (B=32, T=4096, D=U=512):
    q_proj = query @ W1 + b1                      [B, 1, U]
    v_proj = values @ W2 + b2                     [B, T, U]
    scores = tanh(q_proj + v_proj) @ V + bv       [B, T, 1]
    attn   = softmax(scores, axis=1)
    out    = sum(attn * values, axis=1)           [B, D]

Device strategy (per core, 4 batches), using only PE + ACT + DMA (the DVE and
the accum-out paths are unusable on this runtime):
  - Host folds b1/b2 into q_eff = query@W1 + b1 + b2, drops bv (softmax shift
    invariance), ships values twice: natural [T, D] bf16 (context matmul) and
    transposed [D, T] fp8 e4m3 (projection matmul) so the device reads each
    element once and never transposes.
  - v_proj computed transposed [U, t] with W2 stationary, fp8 DoubleRow
    (2 matmuls of K=256) with W2 pre-scaled by F8_SCALE on host, un-scaled
    inside the ACT tanh (scale=1/F8_SCALE); q_eff rides the tanh per-partition
    bias for free.
  - tanh tiles emitted as fp8 e3m4 (4-bit mantissa covers [-1,1] well) and
    kept stationary in the scores matmul so scores land partition-major in
    PSUM. This phase is LDWEIGHTS-stream-bound (T*U elements through the
    1.2 GHz weight port, FWL 2x) - measured no cheaper than any row-major
    DoubleRow + transpose scheme once the transpose cost is counted, and it
    needs no cross-partition moves. V is converted to e3m4 on-device,
    pre-scaled by VC_SCALE to clear the e3m4 subnormal range; exp un-scales.
  - softmax without division or max-subtraction (scores stay O(1), safe in
    fp32): unnormalized attn = exp(s); the host divides by the shipped sum.
    Per-block colsums ride spare columns of the scores PSUM bank.
  - Context: T/128 accumulating [128,1]x[128,512] bf16 matmuls, attn stationary.
  - Flat software pipeline over the 16 (batch, pair) stages: stage s emits
    proj(s) ub-groups with the first half of ctx(s-2) interleaved (pure slack
    that covers the vp PSUM reuse WAR on this stage's tanh), then scores(s-1)
    after proj23 (past the tanh(s-1) tail), then exp(s-1) and the rest of
    ctx(s-2); per-batch tails trail two stages behind. vT is prefetched one
    batch ahead; vN rides one stage behind its batch so the ramp DMA
    bandwidth all goes to the first vT tile.
"""

import os
import sys

import numpy as np

try:
    import ml_dtypes  # noqa: F401
except ImportError:  # pragma: no cover
    sys.path.insert(0, "/opt/trn_rl_repo")
    import ml_dtypes  # noqa: F401

try:
    import concourse  # noqa: F401
except ImportError:  # pragma: no cover
    sys.path.insert(0, "/opt/trn_rl_repo")

BF16 = np.dtype(ml_dtypes.bfloat16)
FP8 = np.dtype(ml_dtypes.float8_e4m3)

B, T, D, U = 32, 4096, 512, 512
N_CORES = 8
BPC = B // N_CORES  # batches per core

F8_SCALE = 64.0  # host scales W2 by this; ACT tanh un-scales via scale=1/F8_SCALE
VC_SCALE = 128.0  # device scales V by this into e3m4; ACT exp un-scales

MODE = os.environ.get("BAHDANAU_MODE", "fp8")

_MODULES: dict = {}


def _build(bpc: int = BPC, t: int = T, mode: str = "fp8"):
    """Build + compile the per-core Bass module. Shapes are per-core shards."""
    from contextlib import ExitStack

    import concourse.bass as bass
    import concourse.tile as tile
    from concourse import bacc, mybir

    f32 = mybir.dt.float32
    bf16 = mybir.dt.bfloat16
    fp8e4 = mybir.dt.float8e4
    fp8e3 = mybir.dt.float8e3
    FT = mybir.ActivationFunctionType
    PSUM = bass.MemorySpace.PSUM
    DR = mybir.MatmulPerfMode.DoubleRow

    tb_n = t // 128  # 128-row t-blocks per batch (32)
    n_pairs = t // 1024  # 1024-wide t-pair stages per batch (4)
    n_stages = bpc * n_pairs

    nc = bacc.Bacc(
        "TRN2", target_bir_lowering=False, debug=False, enable_asserts=False
    )

    vT_d = nc.dram_tensor("valuesT", [bpc, D, t], fp8e4, kind="ExternalInput")
    vN_d = nc.dram_tensor("valuesN", [bpc, t, D], bf16, kind="ExternalInput")
    w2_d = nc.dram_tensor("w2t", [D, U], fp8e4, kind="ExternalInput")
    vc_d = nc.dram_tensor("v_col", [U, 1], bf16, kind="ExternalInput")
    qe_d = nc.dram_tensor("q_eff", [bpc, U], f32, kind="ExternalInput")
    c1b_d = nc.dram_tensor("c_ones_bf", [128, 1], bf16, kind="ExternalInput")
    out_d = nc.dram_tensor("ctx_out", [bpc, D], f32, kind="ExternalOutput")
    cols_d = nc.dram_tensor("colsums", [bpc, tb_n], f32, kind="ExternalOutput")

    with tile.TileContext(nc) as tc, ExitStack() as ctx:
        const = ctx.enter_context(tc.tile_pool(name="const", bufs=1))
        vT_pool = ctx.enter_context(tc.tile_pool(name="vT", bufs=3))
        vN_pool = ctx.enter_context(tc.tile_pool(name="vN", bufs=3))
        tanh_pool = ctx.enter_context(tc.tile_pool(name="tanh", bufs=8))
        sm_pool = ctx.enter_context(tc.tile_pool(name="sm", bufs=2))
        attn_pool = ctx.enter_context(tc.tile_pool(name="attn", bufs=2))
        ctxs_pool = ctx.enter_context(tc.tile_pool(name="ctxs", bufs=2))
        vp_psum = ctx.enter_context(tc.tile_pool(name="vp_ps", bufs=2, space=PSUM))
        sco_psum = ctx.enter_context(tc.tile_pool(name="sc_ps", bufs=2, space=PSUM))
        ctx_psum = ctx.enter_context(tc.tile_pool(name="ctx_ps", bufs=2, space=PSUM))

        # Issue order on the sync ring gates the ramp (~0.8us per dma_start):
        # the first proj matmul needs only vT(0) chunk 0 + w2, so those two go
        # first; the other consts follow the second vT chunk.
        vT0_sb = vT_pool.tile([128, 4, t], fp8e4, name="vT0_sb")
        vT0_src = vT_d[0].rearrange("(db p) tt -> p db tt", p=128)
        nc.sync.dma_start(vT0_sb[:, :, 0:512], vT0_src[:, :, 0:512])
        w2_sb = const.tile([128, 4, U], fp8e4)
        nc.sync.dma_start(w2_sb[:], w2_d.ap().rearrange("(db p) u -> p db u", p=128))
        nc.sync.dma_start(vT0_sb[:, :, 512:1024], vT0_src[:, :, 512:1024])
        vc_sb = const.tile([128, 4], bf16)
        nc.sync.dma_start(
            vc_sb[:], vc_d.ap().rearrange("(ub p) one -> p (ub one)", p=128)
        )
        vc_e3 = const.tile([128, 4], fp8e3)
        nc.scalar.mul(vc_e3[:], vc_sb[:], VC_SCALE)
        qe_sb = const.tile([128, bpc, 4], f32)
        nc.sync.dma_start(qe_sb[:], qe_d.ap().rearrange("b (ub p) -> p b ub", p=128))
        c1b_sb = const.tile([128, 1], bf16)
        nc.sync.dma_start(c1b_sb[:], c1b_d.ap())

        bst: dict[int, dict] = {}  # per-batch live state

        def load_vT(b, chunk, vT_sb=None, lo=0):
            if vT_sb is None:
                vT_sb = vT_pool.tile([128, 4, t], fp8e4, name="vT_sb")
            vT_src = vT_d[b].rearrange("(db p) tt -> p db tt", p=128)
            for c in range(lo // chunk, t // chunk):
                sl = slice(c * chunk, (c + 1) * chunk)
                nc.sync.dma_start(vT_sb[:, :, sl], vT_src[:, :, sl])
            return vT_sb

        def open_batch(b, vT_sb):
            bst[b] = {
                "vT": vT_sb,
                "vN": vN_pool.tile([128, tb_n, D], bf16, name="vN_sb"),
                "vN_src": vN_d[b].rearrange("(n p) dd -> p n dd", p=128),
                "exp": attn_pool.tile([128, tb_n], bf16, name="expP"),
                "th": {},  # pair -> [4 tanh tiles]
            }

        def emit_proj_ub(b, pair, ub):
            """4 DR matmuls + 1 tanh->e3m4 for one (pair, ub) group."""
            st = bst[b]
            vp = vp_psum.tile([128, 2, 512], f32, name="vp")
            # j outer / half inner so consecutive matmuls share the same
            # stationary W2 block (LDWEIGHTS amortization)
            for j in range(2):
                for half in range(2):
                    tc8 = pair * 2 + half
                    nc.tensor.matmul(
                        vp[:, half, :],
                        w2_sb[:, 2 * j : 2 * j + 2, bass.ts(ub, 128)],
                        st["vT"][:, 2 * j : 2 * j + 2, bass.ts(tc8, 512)],
                        start=(j == 0),
                        stop=(j == 1),
                        perf_mode=DR,
                    )
            th = tanh_pool.tile([128, 2, 512], fp8e3, name="th")
            nc.scalar.activation(
                th[:],
                vp[:],
                FT.Tanh,
                bias=qe_sb[:, b, ub : ub + 1],
                scale=1.0 / F8_SCALE,
            )
            st["th"].setdefault(pair, []).append(th)

        def emit_scores(b, pair, lo, hi):
            """scores matmuls for t-blocks [lo, hi) of this pair; tanh tiles
            stationary so the scores land partition-major."""
            st = bst[b]
            if "sco" not in st:
                # lazy: the previous batch's tile (same 2-buffer pool) is read
                # by its exp until one stage after this batch opens.
                # cols 0:tb_n partition-major scores; cols 40:40+tb_n on
                # partition 0 take the ones-matmul colsums row.
                st["sco"] = sco_psum.tile([128, 40 + tb_n], f32, name="scoresP")
            tiles = st["th"][pair]
            for tl8 in range(lo, hi):
                blk = pair * 8 + tl8
                for ub in range(4):
                    nc.tensor.matmul(
                        st["sco"][:, blk : blk + 1],
                        tiles[ub][:, tl8 // 4, bass.ts(tl8 % 4, 128)],
                        vc_e3[:, ub : ub + 1],
                        start=(ub == 0),
                        stop=(ub == 3),
                    )

        def emit_exp(b, pair):
            st = bst[b]
            sl = slice(pair * 8, (pair + 1) * 8)
            nc.scalar.activation(
                st["exp"][:, sl], st["sco"][:, sl], FT.Exp, scale=1.0 / VC_SCALE
            )
            del st["th"][pair]

        def emit_ctx(s, lo=0, hi=8):
            b, pair = divmod(s, n_pairs)
            st = bst[b]
            if "cps" not in st:
                # lazy for the same pool-rotation reason as "sco"
                st["cps"] = ctx_psum.tile([1, D], f32, name="cps")
            for k in range(lo, hi):
                n = pair * 8 + k
                nc.tensor.matmul(
                    st["cps"][:],
                    st["exp"][:, n : n + 1],
                    st["vN"][:, n, :],
                    start=(n == 0),
                    stop=(n == tb_n - 1),
                )

        def emit_tail(b):
            """colsums row + context copy + output DMAs for a finished batch."""
            st = bst.pop(b)
            nc.tensor.matmul(
                st["sco"][0:1, 40 : 40 + tb_n],
                c1b_sb[:],
                st["exp"][:],
                start=True,
                stop=True,
            )
            s1 = sm_pool.tile([1, tb_n], f32)
            nc.scalar.copy(s1[:], st["sco"][0:1, 40 : 40 + tb_n])
            nc.sync.dma_start(cols_d[b : b + 1, :], s1[:])
            cs_raw = ctxs_pool.tile([1, D], f32)
            nc.scalar.copy(cs_raw[:], st["cps"][:])
            nc.sync.dma_start(out_d[b : b + 1, :], cs_raw[:])

        # vN(b, p) is needed by ctx(b, p) at stage 4b+p+2; issue one stage
        # ahead. Exception: vN(0, 0) waits until stage 2 so the ramp's DMA
        # bandwidth all goes to vT(0), which gates the first projections.
        vn_sched: dict[int, list] = {}
        for vb in range(bpc):
            for vp_ in range(n_pairs):
                sched = 2 if (vb, vp_) == (0, 0) else 4 * vb + vp_ + 1
                vn_sched.setdefault(min(sched, n_stages - 1), []).append((vb, vp_))

        # chunks 0-1 were issued up front, ahead of the consts
        vT_next = load_vT(0, 512, vT_sb=vT0_sb, lo=1024)
        for s in range(n_stages):
            b, pair = divmod(s, n_pairs)
            if pair == 0:
                open_batch(b, vT_next)
            st = bst[b]
            if pair == 2 and b + 1 < bpc:
                vT_next = load_vT(b + 1, 1024)
            for vb, vp_ in vn_sched.get(s, ()):
                src = bst[vb]
                nc.sync.dma_start(
                    src["vN"][:, vp_ * 8 : (vp_ + 1) * 8, :],
                    src["vN_src"][:, vp_ * 8 : (vp_ + 1) * 8, :],
                )
            # ctx(s-2) (no fresh deps) is split around proj23 to cover the vp
            # PSUM reuse WAR on this stage's tanh; scores(s-1) sit after
            # proj23, past the tanh(s-1, ub3) tail. Batch 0's ctx stays late:
            # its vN DMAs have less lead time.
            pb, pp = divmod(s - 1, n_pairs) if s >= 1 else (None, None)
            ctx_early = s >= 2 and (s - 2) // n_pairs > 0
            emit_proj_ub(b, pair, 0)
            emit_proj_ub(b, pair, 1)
            if ctx_early:
                emit_ctx(s - 2, 0, 4)
            emit_proj_ub(b, pair, 2)
            emit_proj_ub(b, pair, 3)
            if pb is not None:
                emit_scores(pb, pp, 0, 8)
                emit_exp(pb, pp)
            if s >= 2:
                emit_ctx(s - 2, 4 if ctx_early else 0, 8)
                if (s - 2) % n_pairs == n_pairs - 1:
                    emit_tail((s - 2) // n_pairs)

        # drain: last two stages' scores/exp/ctx + final batch tail
        lb, lp = divmod(n_stages - 1, n_pairs)
        emit_ctx(n_stages - 2)
        emit_scores(lb, lp, 0, 8)
        emit_exp(lb, lp)
        emit_ctx(n_stages - 1)
        emit_tail(lb)

    nc.compile()
    return nc


def _get_module(bpc: int = BPC, t: int = T, mode: str | None = None):
    mode = MODE if mode is None else mode
    key = (mode, bpc, t)
    if key not in _MODULES:
        _MODULES[key] = _build(bpc, t, mode)
    return _MODULES[key]


def _prep_inputs(query, values, W1, b1, W2, b2, V, bv, mode: str | None = None):
    """Host-side preprocessing: fold biases, cast, transpose, shard."""
    query = np.asarray(query, np.float32)
    values = np.asarray(values, np.float32)
    W1 = np.asarray(W1, np.float32)
    b1 = np.asarray(b1, np.float32)
    W2 = np.asarray(W2, np.float32)
    b2 = np.asarray(b2, np.float32)
    V = np.asarray(V, np.float32)

    q_eff = (
        query.astype(np.float64) @ W1.astype(np.float64)
        + b1.astype(np.float64)
        + b2.astype(np.float64)
    ).astype(np.float32)  # [B, U]; bv dropped (softmax shift invariance)

    vN = values.astype(BF16)  # [B, T, D]
    vT = np.ascontiguousarray(values.transpose(0, 2, 1)).astype(FP8)  # [B, D, T]
    w2 = (W2 * F8_SCALE).astype(FP8)
    vcol = np.ascontiguousarray(V.reshape(U, 1)).astype(BF16)
    c1b = np.ones((128, 1), BF16)

    in_maps = []
    for c in range(N_CORES):
        s = slice(c * BPC, (c + 1) * BPC)
        in_maps.append(
            {
                "valuesT": vT[s],
                "valuesN": vN[s],
                "w2t": w2,
                "v_col": vcol,
                "q_eff": q_eff[s],
                "c_ones_bf": c1b,
            }
        )
    return in_maps


def _run(in_maps, trace=False, mode: str | None = None, **kw):
    from concourse.bass_utils import run_bass_kernel_spmd

    nc = _get_module(mode=mode)
    res = run_bass_kernel_spmd(
        nc, in_maps, core_ids=list(range(N_CORES)), trace=trace, **kw
    )
    raw = np.concatenate(
        [np.asarray(res.results[c]["ctx_out"]) for c in range(N_CORES)], axis=0
    ).astype(np.float32)
    sums = np.concatenate(
        [np.asarray(res.results[c]["colsums"]) for c in range(N_CORES)], axis=0
    ).astype(np.float32)
    out = raw / sums.sum(axis=1, keepdims=True)
    return out, res


def kernel(query, values, W1, b1, W2, b2, V, bv):
    in_maps = _prep_inputs(query, values, W1, b1, W2, b2, V, bv)
    out, _ = _run(in_maps, trace=False)
    return out


# revision 36
# speedup vs baseline: 1.0065x; 1.0065x over previous
"""Bahdanau attention kernel for Trainium2 (8 NeuronCores, data-parallel over batch).

Reference computation (B=32, T=4096, D=U=512):
    q_proj = query @ W1 + b1                      [B, 1, U]
    v_proj = values @ W2 + b2                     [B, T, U]
    scores = tanh(q_proj + v_proj) @ V + bv       [B, T, 1]
    attn   = softmax(scores, axis=1)
    out    = sum(attn * values, axis=1)           [B, D]

Device strategy (per core, 4 batches), using only PE + ACT + DMA (the DVE and
the accum-out paths are unusable on this runtime):
  - Host folds b1/b2 into q_eff = query@W1 + b1 + b2, drops bv (softmax shift
    invariance), ships values twice: natural [T, D] bf16 (context matmul) and
    transposed [D, T] fp8 e4m3 (projection matmul) so the device reads each
    element once and never transposes.
  - v_proj computed transposed [U, t] with W2 stationary, fp8 DoubleRow
    (2 matmuls of K=256) with W2 pre-scaled by F8_SCALE on host, un-scaled
    inside the ACT tanh (scale=1/F8_SCALE); q_eff rides the tanh per-partition
    bias for free.
  - tanh tiles emitted as fp8 e3m4 (4-bit mantissa covers [-1,1] well) and
    kept stationary in the scores matmul so scores land partition-major in
    PSUM. This phase is LDWEIGHTS-stream-bound (T*U elements through the
    1.2 GHz weight port, FWL 2x) - measured no cheaper than any row-major
    DoubleRow + transpose scheme once the transpose cost is counted, and it
    needs no cross-partition moves. V is converted to e3m4 on-device,
    pre-scaled by VC_SCALE to clear the e3m4 subnormal range; exp un-scales.
  - softmax without division or max-subtraction (scores stay O(1), safe in
    fp32): unnormalized attn = exp(s); the host divides by the shipped sum.
    Per-block colsums ride spare columns of the scores PSUM bank.
  - Context: T/128 accumulating [128,1]x[128,512] bf16 matmuls, attn stationary.
  - Flat software pipeline over the 16 (batch, pair) stages: stage s emits
    proj(s) ub-groups with the first half of ctx(s-2) interleaved (pure slack
    that covers the vp PSUM reuse WAR on this stage's tanh), then scores(s-1)
    after proj23 (past the tanh(s-1) tail), then exp(s-1) and the rest of
    ctx(s-2); per-batch tails trail two stages behind. vT is prefetched one
    batch ahead; vN rides one stage behind its batch so the ramp DMA
    bandwidth all goes to the first vT tile.
"""

import os
import sys

import numpy as np

try:
    import ml_dtypes  # noqa: F401
except ImportError:  # pragma: no cover
    sys.path.insert(0, "/opt/trn_rl_repo")
    import ml_dtypes  # noqa: F401

try:
    import concourse  # noqa: F401
except ImportError:  # pragma: no cover
    sys.path.insert(0, "/opt/trn_rl_repo")

BF16 = np.dtype(ml_dtypes.bfloat16)
FP8 = np.dtype(ml_dtypes.float8_e4m3)

B, T, D, U = 32, 4096, 512, 512
N_CORES = 8
BPC = B // N_CORES  # batches per core

F8_SCALE = 64.0  # host scales W2 by this; ACT tanh un-scales via scale=1/F8_SCALE
VC_SCALE = 128.0  # device scales V by this into e3m4; ACT exp un-scales

MODE = os.environ.get("BAHDANAU_MODE", "fp8")

_MODULES: dict = {}


def _build(bpc: int = BPC, t: int = T, mode: str = "fp8"):
    """Build + compile the per-core Bass module. Shapes are per-core shards."""
    from contextlib import ExitStack

    import concourse.bass as bass
    import concourse.tile as tile
    from concourse import bacc, mybir

    f32 = mybir.dt.float32
    bf16 = mybir.dt.bfloat16
    fp8e4 = mybir.dt.float8e4
    fp8e3 = mybir.dt.float8e3
    FT = mybir.ActivationFunctionType
    PSUM = bass.MemorySpace.PSUM
    DR = mybir.MatmulPerfMode.DoubleRow

    tb_n = t // 128  # 128-row t-blocks per batch (32)
    n_pairs = t // 1024  # 1024-wide t-pair stages per batch (4)
    n_stages = bpc * n_pairs

    nc = bacc.Bacc(
        "TRN2", target_bir_lowering=False, debug=False, enable_asserts=False
    )

    vT_d = nc.dram_tensor("valuesT", [bpc, D, t], fp8e4, kind="ExternalInput")
    vN_d = nc.dram_tensor("valuesN", [bpc, t, D], bf16, kind="ExternalInput")
    w2_d = nc.dram_tensor("w2t", [D, U], fp8e4, kind="ExternalInput")
    vc_d = nc.dram_tensor("v_col", [U, 1], bf16, kind="ExternalInput")
    qe_d = nc.dram_tensor("q_eff", [bpc, U], f32, kind="ExternalInput")
    c1b_d = nc.dram_tensor("c_ones_bf", [128, 1], bf16, kind="ExternalInput")
    out_d = nc.dram_tensor("ctx_out", [bpc, D], f32, kind="ExternalOutput")
    cols_d = nc.dram_tensor("colsums", [bpc, tb_n], f32, kind="ExternalOutput")

    with tile.TileContext(nc) as tc, ExitStack() as ctx:
        const = ctx.enter_context(tc.tile_pool(name="const", bufs=1))
        vT_pool = ctx.enter_context(tc.tile_pool(name="vT", bufs=3))
        vN_pool = ctx.enter_context(tc.tile_pool(name="vN", bufs=3))
        tanh_pool = ctx.enter_context(tc.tile_pool(name="tanh", bufs=8))
        sm_pool = ctx.enter_context(tc.tile_pool(name="sm", bufs=2))
        attn_pool = ctx.enter_context(tc.tile_pool(name="attn", bufs=2))
        ctxs_pool = ctx.enter_context(tc.tile_pool(name="ctxs", bufs=2))
        vp_psum = ctx.enter_context(tc.tile_pool(name="vp_ps", bufs=2, space=PSUM))
        sco_psum = ctx.enter_context(tc.tile_pool(name="sc_ps", bufs=2, space=PSUM))
        ctx_psum = ctx.enter_context(tc.tile_pool(name="ctx_ps", bufs=2, space=PSUM))

        # Issue order on the sync ring gates the ramp (~0.8us per dma_start):
        # the first proj matmul needs only vT(0) chunk 0 + w2, so those two go
        # first; the other consts follow the second vT chunk.
        vT0_sb = vT_pool.tile([128, 4, t], fp8e4, name="vT0_sb")
        vT0_src = vT_d[0].rearrange("(db p) tt -> p db tt", p=128)
        nc.sync.dma_start(vT0_sb[:, :, 0:512], vT0_src[:, :, 0:512])
        w2_sb = const.tile([128, 4, U], fp8e4)
        nc.sync.dma_start(w2_sb[:], w2_d.ap().rearrange("(db p) u -> p db u", p=128))
        nc.sync.dma_start(vT0_sb[:, :, 512:1024], vT0_src[:, :, 512:1024])
        vc_sb = const.tile([128, 4], bf16)
        nc.sync.dma_start(
            vc_sb[:], vc_d.ap().rearrange("(ub p) one -> p (ub one)", p=128)
        )
        vc_e3 = const.tile([128, 4], fp8e3)
        nc.scalar.mul(vc_e3[:], vc_sb[:], VC_SCALE)
        qe_sb = const.tile([128, bpc, 4], f32)
        nc.sync.dma_start(qe_sb[:], qe_d.ap().rearrange("b (ub p) -> p b ub", p=128))
        c1b_sb = const.tile([128, 1], bf16)
        nc.sync.dma_start(c1b_sb[:], c1b_d.ap())

        bst: dict[int, dict] = {}  # per-batch live state

        def load_vT(b, chunk, vT_sb=None, lo=0):
            if vT_sb is None:
                vT_sb = vT_pool.tile([128, 4, t], fp8e4, name="vT_sb")
            vT_src = vT_d[b].rearrange("(db p) tt -> p db tt", p=128)
            for c in range(lo // chunk, t // chunk):
                sl = slice(c * chunk, (c + 1) * chunk)
                nc.sync.dma_start(vT_sb[:, :, sl], vT_src[:, :, sl])
            return vT_sb

        def open_batch(b, vT_sb):
            bst[b] = {
                "vT": vT_sb,
                "vN": vN_pool.tile([128, tb_n, D], bf16, name="vN_sb"),
                "vN_src": vN_d[b].rearrange("(n p) dd -> p n dd", p=128),
                "exp": attn_pool.tile([128, tb_n], bf16, name="expP"),
                "th": {},  # pair -> [4 tanh tiles]
            }

        def emit_proj_ub(b, pair, ub):
            """4 DR matmuls + 1 tanh->e3m4 for one (pair, ub) group."""
            st = bst[b]
            vp = vp_psum.tile([128, 2, 512], f32, name="vp")
            # j outer / half inner so consecutive matmuls share the same
            # stationary W2 block (LDWEIGHTS amortization)
            for j in range(2):
                for half in range(2):
                    tc8 = pair * 2 + half
                    nc.tensor.matmul(
                        vp[:, half, :],
                        w2_sb[:, 2 * j : 2 * j + 2, bass.ts(ub, 128)],
                        st["vT"][:, 2 * j : 2 * j + 2, bass.ts(tc8, 512)],
                        start=(j == 0),
                        stop=(j == 1),
                        perf_mode=DR,
                    )
            th = tanh_pool.tile([128, 2, 512], fp8e3, name="th")
            nc.scalar.activation(
                th[:],
                vp[:],
                FT.Tanh,
                bias=qe_sb[:, b, ub : ub + 1],
                scale=1.0 / F8_SCALE,
            )
            st["th"].setdefault(pair, []).append(th)

        def emit_scores(b, pair, lo, hi):
            """scores matmuls for t-blocks [lo, hi) of this pair; tanh tiles
            stationary so the scores land partition-major."""
            st = bst[b]
            if "sco" not in st:
                # lazy: the previous batch's tile (same 2-buffer pool) is read
                # by its exp until one stage after this batch opens.
                # cols 0:tb_n partition-major scores; cols 40:40+tb_n on
                # partition 0 take the ones-matmul colsums row.
                st["sco"] = sco_psum.tile([128, 40 + tb_n], f32, name="scoresP")
            tiles = st["th"][pair]
            for tl8 in range(lo, hi):
                blk = pair * 8 + tl8
                for ub in range(4):
                    nc.tensor.matmul(
                        st["sco"][:, blk : blk + 1],
                        tiles[ub][:, tl8 // 4, bass.ts(tl8 % 4, 128)],
                        vc_e3[:, ub : ub + 1],
                        start=(ub == 0),
                        stop=(ub == 3),
                    )

        def emit_exp(b, pair):
            st = bst[b]
            sl = slice(pair * 8, (pair + 1) * 8)
            nc.scalar.activation(
                st["exp"][:, sl], st["sco"][:, sl], FT.Exp, scale=1.0 / VC_SCALE
            )
            del st["th"][pair]

        def emit_ctx(s, lo=0, hi=8):
            b, pair = divmod(s, n_pairs)
            st = bst[b]
            if "cps" not in st:
                # lazy for the same pool-rotation reason as "sco"
                st["cps"] = ctx_psum.tile([1, D], f32, name="cps")
            for k in range(lo, hi):
                n = pair * 8 + k
                nc.tensor.matmul(
                    st["cps"][:],
                    st["exp"][:, n : n + 1],
                    st["vN"][:, n, :],
                    start=(n == 0),
                    stop=(n == tb_n - 1),
                )

        def emit_tail(b):
            """colsums row + context copy + output DMAs for a finished batch."""
            st = bst.pop(b)
            nc.tensor.matmul(
                st["sco"][0:1, 40 : 40 + tb_n],
                c1b_sb[:],
                st["exp"][:],
                start=True,
                stop=True,
            )
            s1 = sm_pool.tile([1, tb_n], f32)
            nc.scalar.copy(s1[:], st["sco"][0:1, 40 : 40 + tb_n])
            nc.sync.dma_start(cols_d[b : b + 1, :], s1[:])
            cs_raw = ctxs_pool.tile([1, D], f32)
            nc.scalar.copy(cs_raw[:], st["cps"][:])
            nc.sync.dma_start(out_d[b : b + 1, :], cs_raw[:])

        # vN(b, p) is needed by ctx(b, p) at stage 4b+p+2; issue one stage
        # ahead. Exception: vN(0, 0) waits until stage 2 so the ramp's DMA
        # bandwidth all goes to vT(0), which gates the first projections.
        vn_sched: dict[int, list] = {}
        for vb in range(bpc):
            for vp_ in range(n_pairs):
                sched = 2 if (vb, vp_) == (0, 0) else 4 * vb + vp_ + 1
                # clamp to n_stages-2: the last batch's final vN transfers
                # otherwise run during the drain, where they contend with the
                # PE instruction-fetch DMA (observed as a token-wait stall)
                vn_sched.setdefault(min(sched, n_stages - 2), []).append((vb, vp_))

        # chunks 0-1 were issued up front, ahead of the consts
        vT_next = load_vT(0, 512, vT_sb=vT0_sb, lo=1024)
        for s in range(n_stages):
            b, pair = divmod(s, n_pairs)
            if pair == 0:
                open_batch(b, vT_next)
            st = bst[b]
            if pair == 2 and b + 1 < bpc:
                vT_next = load_vT(b + 1, 1024)
            for vb, vp_ in vn_sched.get(s, ()):
                src = bst[vb]
                nc.sync.dma_start(
                    src["vN"][:, vp_ * 8 : (vp_ + 1) * 8, :],
                    src["vN_src"][:, vp_ * 8 : (vp_ + 1) * 8, :],
                )
            # ctx(s-2) (no fresh deps) is split around proj23 to cover the vp
            # PSUM reuse WAR on this stage's tanh; scores(s-1) sit after
            # proj23, past the tanh(s-1, ub3) tail. Batch 0's ctx stays late:
            # its vN DMAs have less lead time.
            pb, pp = divmod(s - 1, n_pairs) if s >= 1 else (None, None)
            ctx_early = s >= 2 and (s - 2) // n_pairs > 0
            emit_proj_ub(b, pair, 0)
            emit_proj_ub(b, pair, 1)
            if ctx_early:
                emit_ctx(s - 2, 0, 4)
            emit_proj_ub(b, pair, 2)
            emit_proj_ub(b, pair, 3)
            if pb is not None:
                emit_scores(pb, pp, 0, 8)
                emit_exp(pb, pp)
            if s >= 2:
                emit_ctx(s - 2, 4 if ctx_early else 0, 8)
                if (s - 2) % n_pairs == n_pairs - 1:
                    emit_tail((s - 2) // n_pairs)

        # drain: last two stages' scores/exp/ctx + final batch tail
        lb, lp = divmod(n_stages - 1, n_pairs)
        emit_ctx(n_stages - 2)
        emit_scores(lb, lp, 0, 8)
        emit_exp(lb, lp)
        emit_ctx(n_stages - 1)
        emit_tail(lb)

    nc.compile()
    return nc


def _get_module(bpc: int = BPC, t: int = T, mode: str | None = None):
    mode = MODE if mode is None else mode
    key = (mode, bpc, t)
    if key not in _MODULES:
        _MODULES[key] = _build(bpc, t, mode)
    return _MODULES[key]


def _prep_inputs(query, values, W1, b1, W2, b2, V, bv, mode: str | None = None):
    """Host-side preprocessing: fold biases, cast, transpose, shard."""
    query = np.asarray(query, np.float32)
    values = np.asarray(values, np.float32)
    W1 = np.asarray(W1, np.float32)
    b1 = np.asarray(b1, np.float32)
    W2 = np.asarray(W2, np.float32)
    b2 = np.asarray(b2, np.float32)
    V = np.asarray(V, np.float32)

    q_eff = (
        query.astype(np.float64) @ W1.astype(np.float64)
        + b1.astype(np.float64)
        + b2.astype(np.float64)
    ).astype(np.float32)  # [B, U]; bv dropped (softmax shift invariance)

    vN = values.astype(BF16)  # [B, T, D]
    vT = np.ascontiguousarray(values.transpose(0, 2, 1)).astype(FP8)  # [B, D, T]
    w2 = (W2 * F8_SCALE).astype(FP8)
    vcol = np.ascontiguousarray(V.reshape(U, 1)).astype(BF16)
    c1b = np.ones((128, 1), BF16)

    in_maps = []
    for c in range(N_CORES):
        s = slice(c * BPC, (c + 1) * BPC)
        in_maps.append(
            {
                "valuesT": vT[s],
                "valuesN": vN[s],
                "w2t": w2,
                "v_col": vcol,
                "q_eff": q_eff[s],
                "c_ones_bf": c1b,
            }
        )
    return in_maps


def _run(in_maps, trace=False, mode: str | None = None, **kw):
    from concourse.bass_utils import run_bass_kernel_spmd

    nc = _get_module(mode=mode)
    res = run_bass_kernel_spmd(
        nc, in_maps, core_ids=list(range(N_CORES)), trace=trace, **kw
    )
    raw = np.concatenate(
        [np.asarray(res.results[c]["ctx_out"]) for c in range(N_CORES)], axis=0
    ).astype(np.float32)
    sums = np.concatenate(
        [np.asarray(res.results[c]["colsums"]) for c in range(N_CORES)], axis=0
    ).astype(np.float32)
    out = raw / sums.sum(axis=1, keepdims=True)
    return out, res


def kernel(query, values, W1, b1, W2, b2, V, bv):
    in_maps = _prep_inputs(query, values, W1, b1, W2, b2, V, bv)
    out, _ = _run(in_maps, trace=False)
    return out


# revision 37
# speedup vs baseline: 1.0194x; 1.0128x over previous
"""Bahdanau attention kernel for Trainium2 (8 NeuronCores, data-parallel over batch).

Reference computation (B=32, T=4096, D=U=512):
    q_proj = query @ W1 + b1                      [B, 1, U]
    v_proj = values @ W2 + b2                     [B, T, U]
    scores = tanh(q_proj + v_proj) @ V + bv       [B, T, 1]
    attn   = softmax(scores, axis=1)
    out    = sum(attn * values, axis=1)           [B, D]

Device strategy (per core, 4 batches), using only PE + ACT + DMA (the DVE and
the accum-out paths are unusable on this runtime):
  - Host folds b1/b2 into q_eff = query@W1 + b1 + b2, drops bv (softmax shift
    invariance), ships values twice: natural [T, D] bf16 (context matmul) and
    transposed [D, T] fp8 e4m3 (projection matmul) so the device reads each
    element once and never transposes.
  - v_proj computed transposed [U, t] with W2 stationary, fp8 DoubleRow
    (2 matmuls of K=256) with W2 pre-scaled by F8_SCALE on host, un-scaled
    inside the ACT tanh (scale=1/F8_SCALE); q_eff rides the tanh per-partition
    bias for free.
  - tanh tiles emitted as fp8 e3m4 (4-bit mantissa covers [-1,1] well) and
    kept stationary in the scores matmul so scores land partition-major in
    PSUM. This phase is LDWEIGHTS-stream-bound (T*U elements through the
    1.2 GHz weight port, FWL 2x) - measured no cheaper than any row-major
    DoubleRow + transpose scheme once the transpose cost is counted, and it
    needs no cross-partition moves. V is converted to e3m4 on-device,
    pre-scaled by VC_SCALE to clear the e3m4 subnormal range; exp un-scales.
  - softmax without division or max-subtraction (scores stay O(1), safe in
    fp32): unnormalized attn = exp(s); the host divides by the shipped sum.
    Per-block colsums ride spare columns of the scores PSUM bank.
  - Context: T/128 accumulating [128,1]x[128,512] bf16 matmuls, attn stationary.
  - Flat software pipeline over the 16 (batch, pair) stages: stage s emits
    proj(s) ub-groups with the first half of ctx(s-2) interleaved (pure slack
    that covers the vp PSUM reuse WAR on this stage's tanh), then scores(s-1)
    after proj23 (past the tanh(s-1) tail), then exp(s-1) and the rest of
    ctx(s-2); per-batch tails trail two stages behind. vT is prefetched one
    batch ahead; vN rides one stage behind its batch so the ramp DMA
    bandwidth all goes to the first vT tile.
"""

import os
import sys

import numpy as np

try:
    import ml_dtypes  # noqa: F401
except ImportError:  # pragma: no cover
    sys.path.insert(0, "/opt/trn_rl_repo")
    import ml_dtypes  # noqa: F401

try:
    import concourse  # noqa: F401
except ImportError:  # pragma: no cover
    sys.path.insert(0, "/opt/trn_rl_repo")

BF16 = np.dtype(ml_dtypes.bfloat16)
FP8 = np.dtype(ml_dtypes.float8_e4m3)

B, T, D, U = 32, 4096, 512, 512
N_CORES = 8
BPC = B // N_CORES  # batches per core

F8_SCALE = 64.0  # host scales W2 by this; ACT tanh un-scales via scale=1/F8_SCALE
VC_SCALE = 128.0  # device scales V by this into e3m4; ACT exp un-scales

MODE = os.environ.get("BAHDANAU_MODE", "fp8")

_MODULES: dict = {}


def _build(bpc: int = BPC, t: int = T, mode: str = "fp8"):
    """Build + compile the per-core Bass module. Shapes are per-core shards."""
    from contextlib import ExitStack

    import concourse.bass as bass
    import concourse.tile as tile
    from concourse import bacc, mybir

    f32 = mybir.dt.float32
    bf16 = mybir.dt.bfloat16
    fp8e4 = mybir.dt.float8e4
    fp8e3 = mybir.dt.float8e3
    FT = mybir.ActivationFunctionType
    PSUM = bass.MemorySpace.PSUM
    DR = mybir.MatmulPerfMode.DoubleRow

    tb_n = t // 128  # 128-row t-blocks per batch (32)
    n_pairs = t // 1024  # 1024-wide t-pair stages per batch (4)
    n_stages = bpc * n_pairs

    nc = bacc.Bacc(
        "TRN2", target_bir_lowering=False, debug=False, enable_asserts=False
    )

    vT_d = nc.dram_tensor("valuesT", [bpc, D, t], fp8e4, kind="ExternalInput")
    vN_d = nc.dram_tensor("valuesN", [bpc, t, D], bf16, kind="ExternalInput")
    w2_d = nc.dram_tensor("w2t", [D, U], fp8e4, kind="ExternalInput")
    vc_d = nc.dram_tensor("v_col", [U, 1], bf16, kind="ExternalInput")
    qe_d = nc.dram_tensor("q_eff", [bpc, U], f32, kind="ExternalInput")
    c1b_d = nc.dram_tensor("c_ones_bf", [128, 1], bf16, kind="ExternalInput")
    out_d = nc.dram_tensor("ctx_out", [bpc, D], f32, kind="ExternalOutput")
    cols_d = nc.dram_tensor("colsums", [bpc, tb_n], f32, kind="ExternalOutput")

    with tile.TileContext(nc) as tc, ExitStack() as ctx:
        const = ctx.enter_context(tc.tile_pool(name="const", bufs=1))
        vT_pool = ctx.enter_context(tc.tile_pool(name="vT", bufs=3))
        vN_pool = ctx.enter_context(tc.tile_pool(name="vN", bufs=3))
        tanh_pool = ctx.enter_context(tc.tile_pool(name="tanh", bufs=8))
        sm_pool = ctx.enter_context(tc.tile_pool(name="sm", bufs=2))
        attn_pool = ctx.enter_context(tc.tile_pool(name="attn", bufs=2))
        ctxs_pool = ctx.enter_context(tc.tile_pool(name="ctxs", bufs=2))
        vp_psum = ctx.enter_context(tc.tile_pool(name="vp_ps", bufs=2, space=PSUM))
        sco_psum = ctx.enter_context(tc.tile_pool(name="sc_ps", bufs=2, space=PSUM))
        ctx_psum = ctx.enter_context(tc.tile_pool(name="ctx_ps", bufs=2, space=PSUM))

        # Issue order on the sync ring gates the ramp (~0.8us per dma_start):
        # the first proj matmul needs only vT(0) chunk 0 + w2, so those two go
        # first; the other consts follow the second vT chunk.
        vT0_sb = vT_pool.tile([128, 4, t], fp8e4, name="vT0_sb")
        vT0_src = vT_d[0].rearrange("(db p) tt -> p db tt", p=128)
        nc.sync.dma_start(vT0_sb[:, :, 0:512], vT0_src[:, :, 0:512])
        w2_sb = const.tile([128, 4, U], fp8e4)
        nc.sync.dma_start(w2_sb[:], w2_d.ap().rearrange("(db p) u -> p db u", p=128))
        nc.sync.dma_start(vT0_sb[:, :, 512:1024], vT0_src[:, :, 512:1024])
        vc_sb = const.tile([128, 4], bf16)
        nc.sync.dma_start(
            vc_sb[:], vc_d.ap().rearrange("(ub p) one -> p (ub one)", p=128)
        )
        vc_e3 = const.tile([128, 4], fp8e3)
        nc.scalar.mul(vc_e3[:], vc_sb[:], VC_SCALE)
        qe_sb = const.tile([128, bpc, 4], f32)
        nc.sync.dma_start(qe_sb[:], qe_d.ap().rearrange("b (ub p) -> p b ub", p=128))
        c1b_sb = const.tile([128, 1], bf16)
        nc.sync.dma_start(c1b_sb[:], c1b_d.ap())

        bst: dict[int, dict] = {}  # per-batch live state

        def load_vT(b, chunk, vT_sb=None, lo=0):
            if vT_sb is None:
                vT_sb = vT_pool.tile([128, 4, t], fp8e4, name="vT_sb")
            vT_src = vT_d[b].rearrange("(db p) tt -> p db tt", p=128)
            for c in range(lo // chunk, t // chunk):
                sl = slice(c * chunk, (c + 1) * chunk)
                nc.sync.dma_start(vT_sb[:, :, sl], vT_src[:, :, sl])
            return vT_sb

        def open_batch(b, vT_sb):
            bst[b] = {
                "vT": vT_sb,
                "vN": vN_pool.tile([128, tb_n, D], bf16, name="vN_sb"),
                "vN_src": vN_d[b].rearrange("(n p) dd -> p n dd", p=128),
                "exp": attn_pool.tile([128, tb_n], bf16, name="expP"),
                "th": {},  # pair -> [4 tanh tiles]
            }

        def emit_proj_ub(b, pair, ub):
            """4 DR matmuls + 1 tanh->e3m4 for one (pair, ub) group."""
            st = bst[b]
            vp = vp_psum.tile([128, 2, 512], f32, name="vp")
            # j outer / half inner so consecutive matmuls share the same
            # stationary W2 block (LDWEIGHTS amortization)
            for j in range(2):
                for half in range(2):
                    tc8 = pair * 2 + half
                    nc.tensor.matmul(
                        vp[:, half, :],
                        w2_sb[:, 2 * j : 2 * j + 2, bass.ts(ub, 128)],
                        st["vT"][:, 2 * j : 2 * j + 2, bass.ts(tc8, 512)],
                        start=(j == 0),
                        stop=(j == 1),
                        perf_mode=DR,
                    )
            th = tanh_pool.tile([128, 2, 512], fp8e3, name="th")
            nc.scalar.activation(
                th[:],
                vp[:],
                FT.Tanh,
                bias=qe_sb[:, b, ub : ub + 1],
                scale=1.0 / F8_SCALE,
            )
            st["th"].setdefault(pair, []).append(th)

        def emit_scores(b, pair, lo, hi):
            """scores matmuls for t-blocks [lo, hi) of this pair; tanh tiles
            stationary so the scores land partition-major."""
            st = bst[b]
            if "sco" not in st:
                # lazy: the previous batch's tile (same 2-buffer pool) is read
                # by its exp until one stage after this batch opens.
                # cols 0:tb_n partition-major scores; cols 40:40+tb_n on
                # partition 0 take the ones-matmul colsums row.
                st["sco"] = sco_psum.tile([128, 40 + tb_n], f32, name="scoresP")
            tiles = st["th"][pair]
            for tl8 in range(lo, hi):
                blk = pair * 8 + tl8
                for ub in range(4):
                    nc.tensor.matmul(
                        st["sco"][:, blk : blk + 1],
                        tiles[ub][:, tl8 // 4, bass.ts(tl8 % 4, 128)],
                        vc_e3[:, ub : ub + 1],
                        start=(ub == 0),
                        stop=(ub == 3),
                    )

        def emit_exp(b, pair):
            st = bst[b]
            sl = slice(pair * 8, (pair + 1) * 8)
            nc.scalar.activation(
                st["exp"][:, sl], st["sco"][:, sl], FT.Exp, scale=1.0 / VC_SCALE
            )
            del st["th"][pair]

        def emit_ctx(s, lo=0, hi=8):
            b, pair = divmod(s, n_pairs)
            st = bst[b]
            if "cps" not in st:
                # lazy for the same pool-rotation reason as "sco"
                st["cps"] = ctx_psum.tile([1, D], f32, name="cps")
            for k in range(lo, hi):
                n = pair * 8 + k
                nc.tensor.matmul(
                    st["cps"][:],
                    st["exp"][:, n : n + 1],
                    st["vN"][:, n, :],
                    start=(n == 0),
                    stop=(n == tb_n - 1),
                )

        def emit_tail(b):
            """colsums row + context copy + output DMAs for a finished batch."""
            st = bst.pop(b)
            nc.tensor.matmul(
                st["sco"][0:1, 40 : 40 + tb_n],
                c1b_sb[:],
                st["exp"][:],
                start=True,
                stop=True,
            )
            s1 = sm_pool.tile([1, tb_n], f32)
            nc.scalar.copy(s1[:], st["sco"][0:1, 40 : 40 + tb_n])
            nc.sync.dma_start(cols_d[b : b + 1, :], s1[:])
            cs_raw = ctxs_pool.tile([1, D], f32)
            nc.scalar.copy(cs_raw[:], st["cps"][:])
            nc.sync.dma_start(out_d[b : b + 1, :], cs_raw[:])

        # vN(b, p) is needed by ctx(b, p) at stage 4b+p+2; issue one stage
        # ahead. Exception: vN(0, 0) waits until stage 2 so the ramp's DMA
        # bandwidth all goes to vT(0), which gates the first projections.
        vn_sched: dict[int, list] = {}
        for vb in range(bpc):
            for vp_ in range(n_pairs):
                sched = 2 if (vb, vp_) == (0, 0) else 4 * vb + vp_ + 1
                vn_sched.setdefault(min(sched, n_stages - 1), []).append((vb, vp_))

        # chunks 0-1 were issued up front, ahead of the consts
        vT_next = load_vT(0, 512, vT_sb=vT0_sb, lo=1024)
        for s in range(n_stages):
            b, pair = divmod(s, n_pairs)
            if pair == 0:
                open_batch(b, vT_next)
            st = bst[b]
            if pair == 2 and b + 1 < bpc:
                vT_next = load_vT(b + 1, 1024)
            for vb, vp_ in vn_sched.get(s, ()):
                src = bst[vb]
                nc.sync.dma_start(
                    src["vN"][:, vp_ * 8 : (vp_ + 1) * 8, :],
                    src["vN_src"][:, vp_ * 8 : (vp_ + 1) * 8, :],
                )
            # ctx(s-2) (no fresh deps) is split around proj23 to cover the vp
            # PSUM reuse WAR on this stage's tanh; scores(s-1) sit after
            # proj23, past the tanh(s-1, ub3) tail. Batch 0's ctx stays late:
            # its vN DMAs have less lead time.
            pb, pp = divmod(s - 1, n_pairs) if s >= 1 else (None, None)
            ctx_early = s >= 2 and (s - 2) // n_pairs > 0
            emit_proj_ub(b, pair, 0)
            emit_proj_ub(b, pair, 1)
            if ctx_early:
                emit_ctx(s - 2, 0, 4)
            emit_proj_ub(b, pair, 2)
            emit_proj_ub(b, pair, 3)
            if pb is not None:
                emit_scores(pb, pp, 0, 8)
                emit_exp(pb, pp)
            if s >= 2:
                emit_ctx(s - 2, 4 if ctx_early else 0, 8)
                if (s - 2) % n_pairs == n_pairs - 1:
                    emit_tail((s - 2) // n_pairs)

        # drain: last two stages' scores/exp/ctx + final batch tail
        lb, lp = divmod(n_stages - 1, n_pairs)
        emit_ctx(n_stages - 2)
        emit_scores(lb, lp, 0, 8)
        emit_exp(lb, lp)
        emit_ctx(n_stages - 1)
        emit_tail(lb)

    nc.compile()
    return nc


def _get_module(bpc: int = BPC, t: int = T, mode: str | None = None):
    mode = MODE if mode is None else mode
    key = (mode, bpc, t)
    if key not in _MODULES:
        _MODULES[key] = _build(bpc, t, mode)
    return _MODULES[key]


def _prep_inputs(query, values, W1, b1, W2, b2, V, bv, mode: str | None = None):
    """Host-side preprocessing: fold biases, cast, transpose, shard."""
    query = np.asarray(query, np.float32)
    values = np.asarray(values, np.float32)
    W1 = np.asarray(W1, np.float32)
    b1 = np.asarray(b1, np.float32)
    W2 = np.asarray(W2, np.float32)
    b2 = np.asarray(b2, np.float32)
    V = np.asarray(V, np.float32)

    q_eff = (
        query.astype(np.float64) @ W1.astype(np.float64)
        + b1.astype(np.float64)
        + b2.astype(np.float64)
    ).astype(np.float32)  # [B, U]; bv dropped (softmax shift invariance)

    vN = values.astype(BF16)  # [B, T, D]
    vT = np.ascontiguousarray(values.transpose(0, 2, 1)).astype(FP8)  # [B, D, T]
    w2 = (W2 * F8_SCALE).astype(FP8)
    vcol = np.ascontiguousarray(V.reshape(U, 1)).astype(BF16)
    c1b = np.ones((128, 1), BF16)

    in_maps = []
    for c in range(N_CORES):
        s = slice(c * BPC, (c + 1) * BPC)
        in_maps.append(
            {
                "valuesT": vT[s],
                "valuesN": vN[s],
                "w2t": w2,
                "v_col": vcol,
                "q_eff": q_eff[s],
                "c_ones_bf": c1b,
            }
        )
    return in_maps


def _run(in_maps, trace=False, mode: str | None = None, **kw):
    from concourse.bass_utils import run_bass_kernel_spmd

    nc = _get_module(mode=mode)
    res = run_bass_kernel_spmd(
        nc, in_maps, core_ids=list(range(N_CORES)), trace=trace, **kw
    )
    raw = np.concatenate(
        [np.asarray(res.results[c]["ctx_out"]) for c in range(N_CORES)], axis=0
    ).astype(np.float32)
    sums = np.concatenate(
        [np.asarray(res.results[c]["colsums"]) for c in range(N_CORES)], axis=0
    ).astype(np.float32)
    out = raw / sums.sum(axis=1, keepdims=True)
    return out, res


def kernel(query, values, W1, b1, W2, b2, V, bv):
    in_maps = _prep_inputs(query, values, W1, b1, W2, b2, V, bv)
    out, _ = _run(in_maps, trace=False)
    return out


# revision 38
# speedup vs baseline: 1.0200x; 1.0006x over previous
"""Bahdanau attention kernel for Trainium2 (8 NeuronCores, data-parallel over batch).

Reference computation (B=32, T=4096, D=U=512):
    q_proj = query @ W1 + b1                      [B, 1, U]
    v_proj = values @ W2 + b2                     [B, T, U]
    scores = tanh(q_proj + v_proj) @ V + bv       [B, T, 1]
    attn   = softmax(scores, axis=1)
    out    = sum(attn * values, axis=1)           [B, D]

Device strategy (per core, 4 batches), using only PE + ACT + DMA (the DVE and
the accum-out paths are unusable on this runtime):
  - Host folds b1/b2 into q_eff = query@W1 + b1 + b2, drops bv (softmax shift
    invariance), ships values twice: natural [T, D] bf16 (context matmul) and
    transposed [D, T] fp8 e4m3 (projection matmul) so the device reads each
    element once and never transposes.
  - v_proj computed transposed [U, t] with W2 stationary, fp8 DoubleRow
    (2 matmuls of K=256) with W2 pre-scaled by F8_SCALE on host, un-scaled
    inside the ACT tanh (scale=1/F8_SCALE); q_eff rides the tanh per-partition
    bias for free.
  - tanh tiles emitted as fp8 e3m4 (4-bit mantissa covers [-1,1] well) and
    kept stationary in the scores matmul so scores land partition-major in
    PSUM. This phase is LDWEIGHTS-stream-bound (T*U elements through the
    1.2 GHz weight port, FWL 2x) - measured no cheaper than any row-major
    DoubleRow + transpose scheme once the transpose cost is counted, and it
    needs no cross-partition moves. V is converted to e3m4 on-device,
    pre-scaled by VC_SCALE to clear the e3m4 subnormal range; exp un-scales.
  - softmax without division or max-subtraction (scores stay O(1), safe in
    fp32): unnormalized attn = exp(s); the host divides by the shipped sum.
    Per-block colsums ride spare columns of the scores PSUM bank.
  - Context: T/128 accumulating [128,1]x[128,512] bf16 matmuls, attn stationary.
  - Flat software pipeline over the 16 (batch, pair) stages: stage s emits
    proj(s) ub-groups with the first half of ctx(s-2) interleaved (pure slack
    that covers the vp PSUM reuse WAR on this stage's tanh), then scores(s-1)
    after proj23 (past the tanh(s-1) tail), then exp(s-1) and the rest of
    ctx(s-2); per-batch tails trail two stages behind. vT is prefetched one
    batch ahead; vN rides one stage behind its batch so the ramp DMA
    bandwidth all goes to the first vT tile.
"""

import os
import sys

import numpy as np

try:
    import ml_dtypes  # noqa: F401
except ImportError:  # pragma: no cover
    sys.path.insert(0, "/opt/trn_rl_repo")
    import ml_dtypes  # noqa: F401

try:
    import concourse  # noqa: F401
except ImportError:  # pragma: no cover
    sys.path.insert(0, "/opt/trn_rl_repo")

BF16 = np.dtype(ml_dtypes.bfloat16)
FP8 = np.dtype(ml_dtypes.float8_e4m3)

B, T, D, U = 32, 4096, 512, 512
N_CORES = 8
BPC = B // N_CORES  # batches per core

F8_SCALE = 64.0  # host scales W2 by this; ACT tanh un-scales via scale=1/F8_SCALE
VC_SCALE = 128.0  # device scales V by this into e3m4; ACT exp un-scales

MODE = os.environ.get("BAHDANAU_MODE", "fp8")

_MODULES: dict = {}


def _build(bpc: int = BPC, t: int = T, mode: str = "fp8"):
    """Build + compile the per-core Bass module. Shapes are per-core shards."""
    from contextlib import ExitStack

    import concourse.bass as bass
    import concourse.tile as tile
    from concourse import bacc, mybir

    f32 = mybir.dt.float32
    bf16 = mybir.dt.bfloat16
    fp8e4 = mybir.dt.float8e4
    fp8e3 = mybir.dt.float8e3
    FT = mybir.ActivationFunctionType
    PSUM = bass.MemorySpace.PSUM
    DR = mybir.MatmulPerfMode.DoubleRow

    tb_n = t // 128  # 128-row t-blocks per batch (32)
    n_pairs = t // 1024  # 1024-wide t-pair stages per batch (4)
    n_stages = bpc * n_pairs

    nc = bacc.Bacc(
        "TRN2", target_bir_lowering=False, debug=False, enable_asserts=False
    )

    vT_d = nc.dram_tensor("valuesT", [bpc, D, t], fp8e4, kind="ExternalInput")
    vN_d = nc.dram_tensor("valuesN", [bpc, t, D], bf16, kind="ExternalInput")
    w2_d = nc.dram_tensor("w2t", [D, U], fp8e4, kind="ExternalInput")
    vc_d = nc.dram_tensor("v_col", [U, 1], bf16, kind="ExternalInput")
    qe_d = nc.dram_tensor("q_eff", [bpc, U], f32, kind="ExternalInput")
    c1b_d = nc.dram_tensor("c_ones_bf", [128, 1], bf16, kind="ExternalInput")
    out_d = nc.dram_tensor("ctx_out", [bpc, D], f32, kind="ExternalOutput")
    cols_d = nc.dram_tensor("colsums", [bpc, tb_n], f32, kind="ExternalOutput")

    with tile.TileContext(nc) as tc, ExitStack() as ctx:
        const = ctx.enter_context(tc.tile_pool(name="const", bufs=1))
        vT_pool = ctx.enter_context(tc.tile_pool(name="vT", bufs=3))
        vN_pool = ctx.enter_context(tc.tile_pool(name="vN", bufs=3))
        tanh_pool = ctx.enter_context(tc.tile_pool(name="tanh", bufs=8))
        sm_pool = ctx.enter_context(tc.tile_pool(name="sm", bufs=2))
        attn_pool = ctx.enter_context(tc.tile_pool(name="attn", bufs=2))
        ctxs_pool = ctx.enter_context(tc.tile_pool(name="ctxs", bufs=2))
        vp_psum = ctx.enter_context(tc.tile_pool(name="vp_ps", bufs=2, space=PSUM))
        sco_psum = ctx.enter_context(tc.tile_pool(name="sc_ps", bufs=2, space=PSUM))
        ctx_psum = ctx.enter_context(tc.tile_pool(name="ctx_ps", bufs=2, space=PSUM))

        # Issue order on the sync ring gates the ramp (~0.8us per dma_start):
        # the first proj matmul needs only vT(0) chunk 0 + w2, so those two go
        # first; the other consts follow the second vT chunk.
        vT0_sb = vT_pool.tile([128, 4, t], fp8e4, name="vT0_sb")
        vT0_src = vT_d[0].rearrange("(db p) tt -> p db tt", p=128)
        nc.sync.dma_start(vT0_sb[:, :, 0:512], vT0_src[:, :, 0:512])
        # w2 rides the idle ACT HWDGE ring so its transfer runs concurrently
        # with vT chunk 0 on the sync ring - the first matmul gates on
        # max(w2, chunk0), which serial issue on one ring cannot improve
        w2_sb = const.tile([128, 4, U], fp8e4)
        nc.scalar.dma_start(
            w2_sb[:], w2_d.ap().rearrange("(db p) u -> p db u", p=128)
        )
        nc.sync.dma_start(vT0_sb[:, :, 512:1024], vT0_src[:, :, 512:1024])
        vc_sb = const.tile([128, 4], bf16)
        nc.sync.dma_start(
            vc_sb[:], vc_d.ap().rearrange("(ub p) one -> p (ub one)", p=128)
        )
        vc_e3 = const.tile([128, 4], fp8e3)
        nc.scalar.mul(vc_e3[:], vc_sb[:], VC_SCALE)
        qe_sb = const.tile([128, bpc, 4], f32)
        nc.sync.dma_start(qe_sb[:], qe_d.ap().rearrange("b (ub p) -> p b ub", p=128))
        c1b_sb = const.tile([128, 1], bf16)
        nc.sync.dma_start(c1b_sb[:], c1b_d.ap())

        bst: dict[int, dict] = {}  # per-batch live state

        def load_vT(b, chunk, vT_sb=None, lo=0):
            if vT_sb is None:
                vT_sb = vT_pool.tile([128, 4, t], fp8e4, name="vT_sb")
            vT_src = vT_d[b].rearrange("(db p) tt -> p db tt", p=128)
            for c in range(lo // chunk, t // chunk):
                sl = slice(c * chunk, (c + 1) * chunk)
                nc.sync.dma_start(vT_sb[:, :, sl], vT_src[:, :, sl])
            return vT_sb

        def open_batch(b, vT_sb):
            bst[b] = {
                "vT": vT_sb,
                "vN": vN_pool.tile([128, tb_n, D], bf16, name="vN_sb"),
                "vN_src": vN_d[b].rearrange("(n p) dd -> p n dd", p=128),
                "exp": attn_pool.tile([128, tb_n], bf16, name="expP"),
                "th": {},  # pair -> [4 tanh tiles]
            }

        def emit_proj_ub(b, pair, ub):
            """4 DR matmuls + 1 tanh->e3m4 for one (pair, ub) group."""
            st = bst[b]
            vp = vp_psum.tile([128, 2, 512], f32, name="vp")
            # j outer / half inner so consecutive matmuls share the same
            # stationary W2 block (LDWEIGHTS amortization)
            for j in range(2):
                for half in range(2):
                    tc8 = pair * 2 + half
                    nc.tensor.matmul(
                        vp[:, half, :],
                        w2_sb[:, 2 * j : 2 * j + 2, bass.ts(ub, 128)],
                        st["vT"][:, 2 * j : 2 * j + 2, bass.ts(tc8, 512)],
                        start=(j == 0),
                        stop=(j == 1),
                        perf_mode=DR,
                    )
            th = tanh_pool.tile([128, 2, 512], fp8e3, name="th")
            nc.scalar.activation(
                th[:],
                vp[:],
                FT.Tanh,
                bias=qe_sb[:, b, ub : ub + 1],
                scale=1.0 / F8_SCALE,
            )
            st["th"].setdefault(pair, []).append(th)

        def emit_scores(b, pair, lo, hi):
            """scores matmuls for t-blocks [lo, hi) of this pair; tanh tiles
            stationary so the scores land partition-major."""
            st = bst[b]
            if "sco" not in st:
                # lazy: the previous batch's tile (same 2-buffer pool) is read
                # by its exp until one stage after this batch opens.
                # cols 0:tb_n partition-major scores; cols 40:40+tb_n on
                # partition 0 take the ones-matmul colsums row.
                st["sco"] = sco_psum.tile([128, 40 + tb_n], f32, name="scoresP")
            tiles = st["th"][pair]
            for tl8 in range(lo, hi):
                blk = pair * 8 + tl8
                for ub in range(4):
                    nc.tensor.matmul(
                        st["sco"][:, blk : blk + 1],
                        tiles[ub][:, tl8 // 4, bass.ts(tl8 % 4, 128)],
                        vc_e3[:, ub : ub + 1],
                        start=(ub == 0),
                        stop=(ub == 3),
                    )

        def emit_exp(b, pair):
            st = bst[b]
            sl = slice(pair * 8, (pair + 1) * 8)
            nc.scalar.activation(
                st["exp"][:, sl], st["sco"][:, sl], FT.Exp, scale=1.0 / VC_SCALE
            )
            del st["th"][pair]

        def emit_ctx(s, lo=0, hi=8):
            b, pair = divmod(s, n_pairs)
            st = bst[b]
            if "cps" not in st:
                # lazy for the same pool-rotation reason as "sco"
                st["cps"] = ctx_psum.tile([1, D], f32, name="cps")
            for k in range(lo, hi):
                n = pair * 8 + k
                nc.tensor.matmul(
                    st["cps"][:],
                    st["exp"][:, n : n + 1],
                    st["vN"][:, n, :],
                    start=(n == 0),
                    stop=(n == tb_n - 1),
                )

        def emit_tail(b):
            """colsums row + context copy + output DMAs for a finished batch."""
            st = bst.pop(b)
            nc.tensor.matmul(
                st["sco"][0:1, 40 : 40 + tb_n],
                c1b_sb[:],
                st["exp"][:],
                start=True,
                stop=True,
            )
            s1 = sm_pool.tile([1, tb_n], f32)
            nc.scalar.copy(s1[:], st["sco"][0:1, 40 : 40 + tb_n])
            nc.sync.dma_start(cols_d[b : b + 1, :], s1[:])
            cs_raw = ctxs_pool.tile([1, D], f32)
            nc.scalar.copy(cs_raw[:], st["cps"][:])
            nc.sync.dma_start(out_d[b : b + 1, :], cs_raw[:])

        # vN(b, p) is needed by ctx(b, p) at stage 4b+p+2; issue one stage
        # ahead. Exception: vN(0, 0) waits until stage 2 so the ramp's DMA
        # bandwidth all goes to vT(0), which gates the first projections.
        vn_sched: dict[int, list] = {}
        for vb in range(bpc):
            for vp_ in range(n_pairs):
                sched = 2 if (vb, vp_) == (0, 0) else 4 * vb + vp_ + 1
                vn_sched.setdefault(min(sched, n_stages - 1), []).append((vb, vp_))

        # chunks 0-1 were issued up front, ahead of the consts
        vT_next = load_vT(0, 512, vT_sb=vT0_sb, lo=1024)
        for s in range(n_stages):
            b, pair = divmod(s, n_pairs)
            if pair == 0:
                open_batch(b, vT_next)
            st = bst[b]
            if pair == 2 and b + 1 < bpc:
                vT_next = load_vT(b + 1, 1024)
            for vb, vp_ in vn_sched.get(s, ()):
                src = bst[vb]
                nc.sync.dma_start(
                    src["vN"][:, vp_ * 8 : (vp_ + 1) * 8, :],
                    src["vN_src"][:, vp_ * 8 : (vp_ + 1) * 8, :],
                )
            # ctx(s-2) (no fresh deps) is split around proj23 to cover the vp
            # PSUM reuse WAR on this stage's tanh; scores(s-1) sit after
            # proj23, past the tanh(s-1, ub3) tail. Batch 0's ctx stays late:
            # its vN DMAs have less lead time.
            pb, pp = divmod(s - 1, n_pairs) if s >= 1 else (None, None)
            ctx_early = s >= 2 and (s - 2) // n_pairs > 0
            emit_proj_ub(b, pair, 0)
            emit_proj_ub(b, pair, 1)
            if ctx_early:
                emit_ctx(s - 2, 0, 4)
            emit_proj_ub(b, pair, 2)
            emit_proj_ub(b, pair, 3)
            if pb is not None:
                emit_scores(pb, pp, 0, 8)
                emit_exp(pb, pp)
            if s >= 2:
                emit_ctx(s - 2, 4 if ctx_early else 0, 8)
                if (s - 2) % n_pairs == n_pairs - 1:
                    emit_tail((s - 2) // n_pairs)

        # drain: last two stages' scores/exp/ctx + final batch tail
        lb, lp = divmod(n_stages - 1, n_pairs)
        emit_ctx(n_stages - 2)
        emit_scores(lb, lp, 0, 8)
        emit_exp(lb, lp)
        emit_ctx(n_stages - 1)
        emit_tail(lb)

    nc.compile()
    return nc


def _get_module(bpc: int = BPC, t: int = T, mode: str | None = None):
    mode = MODE if mode is None else mode
    key = (mode, bpc, t)
    if key not in _MODULES:
        _MODULES[key] = _build(bpc, t, mode)
    return _MODULES[key]


def _prep_inputs(query, values, W1, b1, W2, b2, V, bv, mode: str | None = None):
    """Host-side preprocessing: fold biases, cast, transpose, shard."""
    query = np.asarray(query, np.float32)
    values = np.asarray(values, np.float32)
    W1 = np.asarray(W1, np.float32)
    b1 = np.asarray(b1, np.float32)
    W2 = np.asarray(W2, np.float32)
    b2 = np.asarray(b2, np.float32)
    V = np.asarray(V, np.float32)

    q_eff = (
        query.astype(np.float64) @ W1.astype(np.float64)
        + b1.astype(np.float64)
        + b2.astype(np.float64)
    ).astype(np.float32)  # [B, U]; bv dropped (softmax shift invariance)

    vN = values.astype(BF16)  # [B, T, D]
    vT = np.ascontiguousarray(values.transpose(0, 2, 1)).astype(FP8)  # [B, D, T]
    w2 = (W2 * F8_SCALE).astype(FP8)
    vcol = np.ascontiguousarray(V.reshape(U, 1)).astype(BF16)
    c1b = np.ones((128, 1), BF16)

    in_maps = []
    for c in range(N_CORES):
        s = slice(c * BPC, (c + 1) * BPC)
        in_maps.append(
            {
                "valuesT": vT[s],
                "valuesN": vN[s],
                "w2t": w2,
                "v_col": vcol,
                "q_eff": q_eff[s],
                "c_ones_bf": c1b,
            }
        )
    return in_maps


def _run(in_maps, trace=False, mode: str | None = None, **kw):
    from concourse.bass_utils import run_bass_kernel_spmd

    nc = _get_module(mode=mode)
    res = run_bass_kernel_spmd(
        nc, in_maps, core_ids=list(range(N_CORES)), trace=trace, **kw
    )
    raw = np.concatenate(
        [np.asarray(res.results[c]["ctx_out"]) for c in range(N_CORES)], axis=0
    ).astype(np.float32)
    sums = np.concatenate(
        [np.asarray(res.results[c]["colsums"]) for c in range(N_CORES)], axis=0
    ).astype(np.float32)
    out = raw / sums.sum(axis=1, keepdims=True)
    return out, res


def kernel(query, values, W1, b1, W2, b2, V, bv):
    in_maps = _prep_inputs(query, values, W1, b1, W2, b2, V, bv)
    out, _ = _run(in_maps, trace=False)
    return out
